# revision 27
# baseline (speedup 1.0000x reference)
r"""Causal multi-head attention (B=4, T=2048, C=1024, H=16, D=64) on 8 TRN2 NeuronCores.

Sharding: core = (batch b, head-group hg).  b = core // 2, hg = core % 2.
Each core computes, for its batch, the attention-output contribution of its
8 heads; the two cores sharing a batch produce fp16 partial sums of the
o-projection which the host adds together with the (analytically folded)
v-bias/o-bias correction.  k-bias cancels in softmax and is dropped.

Single merged PE stream (matmuls bf16, fp32 PSUM), chunk-ASCENDING:
  prefix (~30us): QK^T projection for query-chunk 0 (all 8 col-blocks) and
  V tiles 0-3, started as soon as the chunk-column xT DMA pieces land.
  main loop over (q-chunk, head-pair) groups: group i's S matmuls + exp
  (pass1, ACT-paced) interleave with group i-1's PV accumulation (pass2);
  the projection matmuls for chunk c+1 (QK^T col-blocks + V tiles) are
  drip-fed as PE filler during chunk c's groups, and o-projection tiles of
  normalized chunks fill the remaining slots.  This keeps ONE uniform
  compute stream: the PE never idles (HAM throttle) and the power draw is
  spread instead of spiking in an exp-heavy phase.
  Group finish is split so the in-order PE queue never waits on the DVE:
    stage1 (right after pass2): evacuate unnormalized O rows, copy the
      denominator rows, reciprocal_approx_fast, cast bf16; frees PSUM.
    stage2 (one group later): broadcast 1/den to 128 partitions with two
      K=1 matmuls (inputs long ready), one full-width normalize multiply.
  Output partials are written as fp16.
"""

import sys

sys.path.insert(0, "/opt/trn_rl_repo")

import numpy as np

import concourse.bass as bass
import concourse.tile as tile
from concourse import bacc, mybir
from concourse.bass_utils import run_bass_kernel_spmd
from concourse.masks import make_upper_triangular

B, T, C = 4, 2048, 1024
H = 16
D = C // H          # 64
HL = 8              # heads per core
HD = HL * D         # 512: local head dim
N_CORES = 8
CB = C // 128       # 8 c-tiles
TQ_CH = T // 512    # 4 query chunks
TK_TILES = T // 128  # 16 key tiles

F32 = mybir.dt.float32
FP8 = mybir.dt.float8e4
PROJ_SCALE = 1.0 / 32768.0
F16 = mybir.dt.float16
BF16 = mybir.dt.bfloat16

_compiled = None
TRACE = False          # set True (e.g. from test.py) to neuron-profile the run
LAST_EXEC_NS = None    # filled with max per-core exec_time_ns when TRACE
LAST_TRACE = None      # (insts, trace_path) when TRACE


def _build():
    nc = bacc.Bacc("TRN2", target_bir_lowering=False, debug=False,
                   num_devices=N_CORES)

    xT_ap = nc.dram_tensor("xT", [C, T], BF16, kind="ExternalInput").ap()
    # fp8(e4m3) copies for the DoubleRow QK projection: x scaled by 16,
    # wqk by 2048 (product 2^15, divided out at PSUM evacuation).  Layout
    # pairs the contraction dim: c = cbp*256 + j*128 + p for (p, j).
    xT8_ap = nc.dram_tensor("xT8", [TQ_CH, 128, 4, 2, 512], FP8,
                            kind="ExternalInput").ap()
    wqk8_ap = nc.dram_tensor("wqk8", [128, 8, 4, 2, 128], FP8,
                             kind="ExternalInput").ap()
    bq_ap = nc.dram_tensor("bq", [4, 128, 1], F32, kind="ExternalInput").ap()
    wv_ap = nc.dram_tensor("wv", [CB, 128, HD], BF16, kind="ExternalInput").ap()
    # wo[g] = rows of Wo for head pair g (head 2g rows 0-63, head 2g+1 rows 64-127)
    wo_ap = nc.dram_tensor("wo", [4, 128, C], BF16, kind="ExternalInput").ap()
    out_ap = nc.dram_tensor("out_p", [T, C], F16, kind="ExternalOutput").ap()

    with tile.TileContext(nc) as tc:
        with (
            tc.tile_pool(name="const", bufs=1) as const_pool,
            tc.tile_pool(name="qkt", bufs=1) as qkt_pool,
            tc.tile_pool(name="v", bufs=1) as v_pool,
            tc.tile_pool(name="ot", bufs=1) as ot_pool,
            tc.tile_pool(name="wo", bufs=1) as wo_pool,
            tc.tile_pool(name="xt", bufs=1) as xt_pool,
            tc.tile_pool(name="wv", bufs=1) as wv_pool,
            tc.tile_pool(name="wqk", bufs=1) as wqk_pool,
            tc.tile_pool(name="e", bufs=2) as e_pool,
            tc.tile_pool(name="e2", bufs=2) as e2_pool,
            tc.tile_pool(name="rb", bufs=1) as rb_pool,
            tc.tile_pool(name="rb2", bufs=2) as rb2_pool,
            tc.tile_pool(name="ps_s", bufs=2, space="PSUM") as ps_s_pool,
            tc.tile_pool(name="ps_o", bufs=1, space="PSUM") as ps_o_pool,
            tc.tile_pool(name="ps_p", bufs=2, space="PSUM") as ps_p_pool,
            tc.tile_pool(name="ostg", bufs=2) as ostg_pool,
        ):
            QKT = [qkt_pool.tile([128, T], BF16, name=f"qkt{n}") for n in range(8)]
            V = [v_pool.tile([128, HL, D + 1], BF16, name=f"v{t}")
                 for t in range(TK_TILES)]
            OT = [ot_pool.tile([128, T], BF16, name=f"ot{g}") for g in range(4)]
            WO = [wo_pool.tile([128, C], BF16, name=f"wo{g}") for g in range(4)]
            WT8h = [wqk_pool.tile([128, 4, 4, 2, 128], FP8, name=f"wt8{h}")
                    for h in range(2)]
            XT8 = [xt_pool.tile([128, 4, 2, 512], FP8, name=f"x8{q}")
                   for q in range(TQ_CH)]
            # bf16 x tiles (V projection lhsT) rotate 2 chunks deep with
            # just-in-time DMA; XTB[c] is allocated at chunk c-1 start.
            XTB = {}

            def alloc_xtb(c, eng):
                XTB[c] = [xt_pool.tile([128, 512], BF16, name=f"xtb{cb}")
                          for cb in range(CB)]
                for cb in range(CB):
                    eng.dma_start(
                        XTB[c][cb][:],
                        xT_ap[cb * 128:(cb + 1) * 128, c * 512:(c + 1) * 512],
                    )
            WV = [wv_pool.tile([128, HD], BF16, name=f"wv{cb}") for cb in range(CB)]

            # DMA queues (only sync/scalar/gpsimd may start DMAs): wqk8
            # on sync; xT8 chunk-0 + bf16-x chunk-0 + wv on scalar; bq +
            # later xT8 chunks + wo on gpsimd.
            nc.sync.dma_start(WT8h[0][:], wqk8_ap[:, 0:4])
            nc.sync.dma_start(WT8h[1][:], wqk8_ap[:, 4:8])
            nc.scalar.dma_start(XT8[0][:], xT8_ap[0])
            alloc_xtb(0, nc.gpsimd)
            bq_t = [const_pool.tile([128, 1], F32, name=f"bq{n}") for n in range(4)]
            for n in range(4):
                nc.gpsimd.dma_start(bq_t[n][:], bq_ap[n])
            for q in range(1, TQ_CH):
                nc.gpsimd.dma_start(XT8[q][:], xT8_ap[q])
            for cb in range(CB):
                nc.scalar.dma_start(WV[cb][:], wv_ap[cb])
            for g in range(4):
                nc.gpsimd.dma_start(WO[g][:], wo_ap[g])

            # constants: triangular mask (valid = key_i <= query_j); [1,128]
            # head-pair broadcast masks ones_a (cols 0-63) / ones_b (64-127).
            trif = const_pool.tile([128, 128], F32)
            make_upper_triangular(nc, trif, val=1.0, diag=True)
            tri = const_pool.tile([128, 128], BF16)
            nc.gpsimd.tensor_copy(tri[:], trif[:])
            oaf = const_pool.tile([1, 128], F32)
            nc.gpsimd.memset(oaf, 0.0)
            nc.gpsimd.memset(oaf[0:1, 0:64], 1.0)
            obf = const_pool.tile([1, 128], F32)
            nc.gpsimd.memset(obf, 0.0)
            nc.gpsimd.memset(obf[0:1, 64:128], 1.0)
            ones_a = const_pool.tile([1, 128], BF16)
            nc.gpsimd.tensor_copy(ones_a[:], oaf[:])
            ones_b = const_pool.tile([1, 128], BF16)
            nc.gpsimd.tensor_copy(ones_b[:], obf[:])
            onesf = const_pool.tile([128, HL], F32)
            nc.gpsimd.memset(onesf, 1.0)
            for t in range(TK_TILES):
                nc.gpsimd.tensor_copy(V[t][:, :, D], onesf[:])

            # ---------- projection / o-projection filler machinery ----------
            fill_q = []          # ('qkt', n, q1) | ('v', tt), item = 8 matmuls

            def emit_fill_item():
                kind = fill_q[0][0]
                ps = ps_p_pool.tile([128, 512], F32, name="psp")
                if kind == 'qkt':
                    _, n, q1 = fill_q.pop(0)
                    for cbp in range(4):
                        nc.tensor.matmul(
                            ps[:], WT8h[n // 4][:, n % 4, cbp],
                            XT8[q1][:, cbp],
                            start=(cbp == 0), stop=(cbp == 3),
                            perf_mode=mybir.MatmulPerfMode.DoubleRow,
                        )
                    dst = QKT[n][:, q1 * 512:(q1 + 1) * 512]
                    if n < 4:
                        nc.vector.tensor_scalar(
                            dst, ps[:], PROJ_SCALE, bq_t[n][:],
                            mybir.AluOpType.mult, mybir.AluOpType.add,
                        )
                    else:
                        nc.vector.tensor_scalar_mul(dst, ps[:], PROJ_SCALE)
                else:
                    _, tt = fill_q.pop(0)
                    q1, c0 = tt // 4, (tt % 4) * 128
                    for cb in range(CB):
                        nc.tensor.matmul(
                            ps[:], XTB[q1][cb][:, c0:c0 + 128], WV[cb][:],
                            start=(cb == 0), stop=(cb == CB - 1),
                        )
                    nc.vector.tensor_copy(
                        V[tt][:, :, 0:D],
                        ps[:].rearrange("p (h d) -> p h d", h=HL),
                    )

            def chunk_items(c):
                # QK col-blocks in first-use order, then the chunk's V tiles
                return ([('qkt', n, c) for n in (0, 4, 1, 5, 2, 6, 3, 7)]
                        + [('v', tt) for tt in range(4 * c, 4 * c + 4)])

            oproj_items = []     # (tt, half) ready once their chunk normalized

            def emit_oproj_item():
                tt, half = oproj_items.pop(0)
                n0 = half * 512
                psp = ps_p_pool.tile([128, 512], F32, name="psp")
                for g in range(4):
                    nc.tensor.matmul(
                        psp[:],
                        OT[g][:, tt * 128:(tt + 1) * 128],
                        WO[g][:, n0:n0 + 512],
                        start=(g == 0), stop=(g == 3),
                    )
                ob = ostg_pool.tile([128, 512], F16, name="ob")
                nc.vector.tensor_copy(ob[:], psp[:])
                nc.sync.dma_start(
                    out_ap[tt * 128:(tt + 1) * 128, n0:n0 + 512], ob[:]
                )

            def stage1(fin):
                """pass2(fin) just ended: evacuate + prep 1/den; frees pso."""
                q0p, hpp, ta, ca, tb, cb2 = fin
                cols = slice(q0p * 512, q0p * 512 + 512)
                nc.vector.tensor_copy(OT[hpp][0:64, cols],
                                      ta[0:64, ca:ca + 512])
                nc.vector.tensor_copy(OT[hpp][64:128, cols],
                                      tb[0:64, cb2:cb2 + 512])
                sd = rb_pool.tile([1, 1024], F32, name="sd")
                nc.vector.tensor_copy(sd[0:1, 0:512], ta[64:65, ca:ca + 512])
                nc.vector.tensor_copy(sd[0:1, 512:1024],
                                      tb[64:65, cb2:cb2 + 512])
                rf = rb_pool.tile([1, 1024], F32, name="rf")
                nc.vector.reciprocal_approx_fast(rf[:], sd[:])
                rb8 = rb2_pool.tile([1, 1024], BF16, name="rb8")
                nc.vector.tensor_copy(rb8[:], rf[:])
                return (q0p, hpp, rb8[0:1, 0:512], rb8[0:1, 512:1024])

            def stage2(fin2):
                """one group later: broadcast 1/den and normalize OT."""
                q0p, hpp, rba, rbb = fin2
                cols = slice(q0p * 512, q0p * 512 + 512)
                psb = ps_p_pool.tile([128, 512], F32, name="psp")
                nc.tensor.matmul(psb[:], ones_a[:], rba[:],
                                 start=True, stop=False)
                nc.tensor.matmul(psb[:], ones_b[:], rbb[:],
                                 start=False, stop=True)
                nc.vector.tensor_mul(OT[hpp][:, cols], OT[hpp][:, cols],
                                     psb[:])
                if hpp == 3:   # chunk q0p fully normalized
                    oproj_items.extend(
                        (tt, half)
                        for tt in range(4 * q0p, 4 * q0p + 4)
                        for half in range(2)
                    )

            cs_total, cs_idx, fill_emitted = 1, 0, 0
            # ---------------- prefix: chunk-0 projection ----------------
            fill_q.extend(chunk_items(0))
            while fill_q:
                emit_fill_item()

            # ---------------- merged main loop (chunks ascending) ----------
            groups = [(q0, hp) for q0 in range(TQ_CH) for hp in range(4)]
            prev = None   # (q0, hp, Ea, Eb) whose pass2 runs this iteration
            fin2 = None   # stage1 output awaiting stage2

            for cur in groups + [None]:
                ntk_c = 4 * (cur[0] + 1) if cur is not None else 0
                ntk_p = 4 * (prev[0] + 1) if prev is not None else 0
                if cur is not None:
                    q0, hp = cur
                    qt, kt = QKT[hp], QKT[4 + hp]
                    tq0 = q0 * 512
                    Ea, Eb = {}, {}
                    if hp == 0:
                        cs_total = 16 * (q0 + 1)
                        cs_idx = 0
                        fill_emitted = 0
                        if q0 + 1 < TQ_CH:
                            alloc_xtb(q0 + 1, nc.gpsimd)
                            fill_q.extend(chunk_items(q0 + 1))
                pso = None
                for j in range(max(ntk_c, ntk_p)):
                    if cur is not None and j < ntk_c:
                        r = j - q0 * 4
                        j0 = r * 128 if r >= 0 else 0
                        pss = ps_s_pool.tile([128, 1024], F32, name="pss")
                        nc.tensor.matmul(
                            pss[:, j0:512],
                            kt[0:64, j * 128:(j + 1) * 128],
                            qt[0:64, tq0 + j0:tq0 + 512],
                            start=True, stop=True,
                        )
                        nc.tensor.matmul(
                            pss[:, 512 + j0:1024],
                            kt[64:128, j * 128:(j + 1) * 128],
                            qt[64:128, tq0 + j0:tq0 + 512],
                            start=True, stop=True,
                        )
                        epool = e_pool if j < 12 else e2_pool
                        e_ab = epool.tile([128, 1024], BF16, name=f"eab{j}")
                        Ea[j] = Eb[j] = e_ab
                        # one wide activation covering both heads' S
                        # (spans both PSUM banks of the pss tile); on
                        # diagonal steps the [512:512+j0] middle is stale
                        # junk that is exp'd but never read
                        nc.scalar.activation(
                            e_ab[:, j0:1024], pss[:, j0:1024],
                            mybir.ActivationFunctionType.Exp,
                            scale=float(D) ** -0.5,
                        )
                        if r >= 0:
                            nc.gpsimd.tensor_mul(
                                e_ab[:, j0:j0 + 128], e_ab[:, j0:j0 + 128],
                                tri[:]
                            )
                            nc.gpsimd.tensor_mul(
                                e_ab[:, 512 + j0:512 + j0 + 128],
                                e_ab[:, 512 + j0:512 + j0 + 128], tri[:]
                            )
                    if cur is not None:
                        cs_idx += 1
                    if fill_q and (cur is None or
                                   fill_emitted * cs_total < cs_idx * 12):
                        emit_fill_item()
                        fill_emitted += 1
                    elif oproj_items:
                        emit_oproj_item()
                    if prev is not None and j < ntk_p:
                        q0p, hpp, pEa, pEb = prev
                        if j == 0:
                            if (q0p, hpp) == groups[-1]:
                                # ps_p banks have been idle since the last
                                # broadcast -> no wait, unlike ps_o/ps_s
                                pso = (
                                    ps_p_pool.tile([128, 512], F32,
                                                   name="psp"), 0,
                                    ps_p_pool.tile([128, 512], F32,
                                                   name="psp"), 0,
                                )
                            else:
                                pso = (
                                    ps_o_pool.tile([65, 512], F32,
                                                   name="pso_a"), 0,
                                    ps_o_pool.tile([65, 512], F32,
                                                   name="pso_b"), 0,
                                )
                        ta, ca, tb, cb2 = pso
                        rp = j - q0p * 4
                        j0p = rp * 128 if rp >= 0 else 0
                        nc.tensor.matmul(
                            ta[0:65, ca + j0p:ca + 512],
                            V[j][:, 2 * hpp, :],
                            pEa[j][:, j0p:512],
                            start=(j == 0), stop=(j == ntk_p - 1),
                        )
                        nc.tensor.matmul(
                            tb[0:65, cb2 + j0p:cb2 + 512],
                            V[j][:, 2 * hpp + 1, :],
                            pEb[j][:, 512 + j0p:1024],
                            start=(j == 0), stop=(j == ntk_p - 1),
                        )
                new_fin2 = None
                if prev is not None:
                    q0p, hpp, _, _ = prev
                    new_fin2 = stage1((q0p, hpp) + pso)
                if fin2 is not None:
                    stage2(fin2)
                fin2 = new_fin2
                prev = (q0, hp, Ea, Eb) if cur is not None else None
            stage2(fin2)
            while oproj_items:
                emit_oproj_item()

    nc.compile()
    return nc


def _prep_core_inputs(hidden_state, qkv_w, qkv_b, o_w, b, hg):
    """Build the per-core input map for batch b, head group hg."""
    import ml_dtypes
    bf16 = ml_dtypes.bfloat16
    s = slice(hg * HD, (hg + 1) * HD)
    wq = qkv_w[:, 0 * C:1 * C][:, s]          # [C, 512]
    wk = qkv_w[:, 1 * C:2 * C][:, s]          # [C, 512]
    wv = qkv_w[:, 2 * C:3 * C][:, s]          # [C, 512]
    bq = qkv_b[0 * C:1 * C][s]                # [512]

    wqk = np.concatenate([wq, wk], axis=1)    # [C, 1024]
    f8 = mybir.dt.np(mybir.dt.float8e4)
    # paired-contraction fp8 layouts: c = cbp*256 + j*128 + p
    wqk8 = np.ascontiguousarray(
        (wqk * 2048.0).reshape(4, 2, 128, 8, 128).transpose(2, 3, 0, 1, 4)
    ).astype(f8)
    bq_r = np.ascontiguousarray(bq.reshape(4, 128, 1))
    wv_r = np.ascontiguousarray(wv.reshape(CB, 128, HD))
    # o_w rows for this head group, regrouped [g, 128, C] in head-pair order
    wo = o_w[hg * HD:(hg + 1) * HD, :]        # [512, C]
    wo_r = np.ascontiguousarray(wo.reshape(4, 128, C))

    xT = np.ascontiguousarray(hidden_state[b].T)  # [C, T]
    xT8 = np.ascontiguousarray(
        (xT * 16.0).reshape(4, 2, 128, TQ_CH, 512).transpose(3, 2, 0, 1, 4)
    ).astype(f8)
    return {
        "xT": xT.astype(bf16),
        "xT8": xT8,
        "wqk8": wqk8,
        "bq": bq_r.astype(np.float32),
        "wv": wv_r.astype(bf16),
        "wo": wo_r.astype(bf16),
    }


def _ensure_profile_hook():
    """Register the NTFF profiling hook that this container's antenv lacks."""
    import types
    try:
        from antenv.axon_hooks import get_axon_ntff_profile_hook  # noqa: F401
        return
    except ImportError:
        pass
    try:
        import antenv
        from trn_agent_boot.trn_boot import _ntff_profile_via_ctypes
        hook = {"h": _ntff_profile_via_ctypes("/opt/axon/libaxon_pjrt.so")}
        mod = types.ModuleType("antenv.axon_hooks")
        mod.set_axon_ntff_profile_hook = lambda h: hook.__setitem__("h", h)
        mod.get_axon_ntff_profile_hook = lambda: hook["h"]
        sys.modules["antenv.axon_hooks"] = mod
        antenv.axon_hooks = mod
    except Exception as e:  # profiling is best-effort
        print(f"profile hook setup failed: {e}", flush=True)


def kernel(hidden_state, qkv_w, qkv_b, o_w, o_b):
    global _compiled
    hidden_state = np.asarray(hidden_state, dtype=np.float32)
    qkv_w = np.asarray(qkv_w, dtype=np.float32)
    qkv_b = np.asarray(qkv_b, dtype=np.float32)
    o_w = np.asarray(o_w, dtype=np.float32)
    o_b = np.asarray(o_b, dtype=np.float32)

    if _compiled is None:
        _compiled = _build()
    nc = _compiled

    in_maps = []
    for core in range(N_CORES):
        b, hg = core // 2, core % 2
        in_maps.append(_prep_core_inputs(hidden_state, qkv_w, qkv_b, o_w, b, hg))

    global LAST_EXEC_NS, LAST_TRACE
    kw = {}
    if TRACE:
        import tempfile
        _ensure_profile_hook()
        kw = dict(trace=True, tmpdir=tempfile.mkdtemp(prefix="bass_attn_trace_"))
    res = run_bass_kernel_spmd(nc, in_maps, core_ids=list(range(N_CORES)), **kw)
    LAST_EXEC_NS = res.exec_time_ns
    LAST_TRACE = res.instructions_and_trace

    # host-side gather: sum the two head-group partials per batch and add the
    # affine correction (v-bias pushed through Wo, plus o-bias).
    bv = qkv_b[2 * C:3 * C]                   # [C]
    corr = (bv @ o_w + o_b).astype(np.float32)
    out = np.empty((B, T, C), dtype=np.float32)
    for b in range(B):
        p0 = res.results[2 * b]["out_p"].astype(np.float32)
        p1 = res.results[2 * b + 1]["out_p"].astype(np.float32)
        out[b] = p0 + p1 + corr
    return out


# revision 28
# speedup vs baseline: 1.1958x; 1.1958x over previous
r"""Causal multi-head attention (B=4, T=2048, C=1024, H=16, D=64) on 8 TRN2 NeuronCores.

Sharding: core = (batch b, head-group hg).  b = core // 2, hg = core % 2.
Each core computes, for its batch, the attention-output contribution of its
8 heads; the two cores sharing a batch produce fp16 partial sums of the
o-projection which the host adds together with the (analytically folded)
v-bias/o-bias correction.  k-bias cancels in softmax and is dropped.

Single merged PE stream (matmuls bf16, fp32 PSUM), chunk-ASCENDING:
  prefix (~30us): QK^T projection for query-chunk 0 (all 8 col-blocks) and
  V tiles 0-3, started as soon as the chunk-column xT DMA pieces land.
  main loop over (q-chunk, head-pair) groups: group i's S matmuls + exp
  (pass1, ACT-paced) interleave with group i-1's PV accumulation (pass2);
  the projection matmuls for chunk c+1 (QK^T col-blocks + V tiles) are
  drip-fed as PE filler during chunk c's groups, and o-projection tiles of
  normalized chunks fill the remaining slots.  This keeps ONE uniform
  compute stream: the PE never idles (HAM throttle) and the power draw is
  spread instead of spiking in an exp-heavy phase.
  Group finish is split so the in-order PE queue never waits on the DVE:
    stage1 (right after pass2): evacuate unnormalized O rows, copy the
      denominator rows, reciprocal_approx_fast, cast bf16; frees PSUM.
    stage2 (one group later): broadcast 1/den to 128 partitions with two
      K=1 matmuls (inputs long ready), one full-width normalize multiply.
  Output partials are written as fp16.
"""

import sys

sys.path.insert(0, "/opt/trn_rl_repo")

import numpy as np

import concourse.bass as bass
import concourse.tile as tile
from concourse import bacc, mybir
from concourse.bass_utils import run_bass_kernel_spmd
from concourse.masks import make_upper_triangular

B, T, C = 4, 2048, 1024
H = 16
D = C // H          # 64
HL = 8              # heads per core
HD = HL * D         # 512: local head dim
N_CORES = 8
CB = C // 128       # 8 c-tiles
TQ_CH = T // 512    # 4 query chunks
TK_TILES = T // 128  # 16 key tiles

F32 = mybir.dt.float32
FP8 = mybir.dt.float8e4
PROJ_SCALE = 1.0 / 32768.0
F16 = mybir.dt.float16
BF16 = mybir.dt.bfloat16

_compiled = None
TRACE = False          # set True (e.g. from test.py) to neuron-profile the run
LAST_EXEC_NS = None    # filled with max per-core exec_time_ns when TRACE
LAST_TRACE = None      # (insts, trace_path) when TRACE


def _build():
    nc = bacc.Bacc("TRN2", target_bir_lowering=False, debug=False,
                   num_devices=N_CORES)

    xT_ap = nc.dram_tensor("xT", [C, T], BF16, kind="ExternalInput").ap()
    # fp8(e4m3) copies for the DoubleRow QK projection: x scaled by 16,
    # wqk by 2048 (product 2^15, divided out at PSUM evacuation).  Layout
    # pairs the contraction dim: c = cbp*256 + j*128 + p for (p, j).
    xT8_ap = nc.dram_tensor("xT8", [TQ_CH, 128, 4, 2, 512], FP8,
                            kind="ExternalInput").ap()
    wqk8_ap = nc.dram_tensor("wqk8", [128, 8, 4, 2, 128], FP8,
                             kind="ExternalInput").ap()
    bq_ap = nc.dram_tensor("bq", [4, 128, 1], F32, kind="ExternalInput").ap()
    wv_ap = nc.dram_tensor("wv", [CB, 128, HD], BF16, kind="ExternalInput").ap()
    # wo[g] = rows of Wo for head pair g (head 2g rows 0-63, head 2g+1 rows 64-127)
    wo_ap = nc.dram_tensor("wo", [4, 128, C], BF16, kind="ExternalInput").ap()
    out_ap = nc.dram_tensor("out_p", [T, C], F16, kind="ExternalOutput").ap()

    with tile.TileContext(nc) as tc:
        with (
            tc.tile_pool(name="const", bufs=1) as const_pool,
            tc.tile_pool(name="qkt", bufs=1) as qkt_pool,
            tc.tile_pool(name="v", bufs=1) as v_pool,
            tc.tile_pool(name="ot", bufs=1) as ot_pool,
            tc.tile_pool(name="wo", bufs=1) as wo_pool,
            tc.tile_pool(name="xt", bufs=1) as xt_pool,
            tc.tile_pool(name="wv", bufs=1) as wv_pool,
            tc.tile_pool(name="wqk", bufs=1) as wqk_pool,
            tc.tile_pool(name="e", bufs=2) as e_pool,
            tc.tile_pool(name="e2", bufs=2) as e2_pool,
            tc.tile_pool(name="rb", bufs=1) as rb_pool,
            tc.tile_pool(name="rb2", bufs=2) as rb2_pool,
            tc.tile_pool(name="ps_s", bufs=2, space="PSUM") as ps_s_pool,
            tc.tile_pool(name="ps_o", bufs=1, space="PSUM") as ps_o_pool,
            tc.tile_pool(name="ps_p", bufs=2, space="PSUM") as ps_p_pool,
            tc.tile_pool(name="ostg", bufs=2) as ostg_pool,
        ):
            QKT = [qkt_pool.tile([128, T], BF16, name=f"qkt{n}") for n in range(8)]
            V = [v_pool.tile([128, HL, D + 1], BF16, name=f"v{t}")
                 for t in range(TK_TILES)]
            OT = [ot_pool.tile([128, T], BF16, name=f"ot{g}") for g in range(4)]
            WO = [wo_pool.tile([128, C], BF16, name=f"wo{g}") for g in range(4)]
            WT8h = [wqk_pool.tile([128, 4, 4, 2, 128], FP8, name=f"wt8{h}")
                    for h in range(2)]
            XT8 = [xt_pool.tile([128, 4, 2, 512], FP8, name=f"x8{q}")
                   for q in range(TQ_CH)]
            # bf16 x tiles (V projection lhsT) rotate 2 chunks deep with
            # just-in-time DMA; XTB[c] is allocated at chunk c-1 start.
            XTB = {}

            def alloc_xtb(c, eng):
                XTB[c] = [xt_pool.tile([128, 512], BF16, name=f"xtb{cb}")
                          for cb in range(CB)]
                for cb in range(CB):
                    eng.dma_start(
                        XTB[c][cb][:],
                        xT_ap[cb * 128:(cb + 1) * 128, c * 512:(c + 1) * 512],
                    )
            WV = [wv_pool.tile([128, HD], BF16, name=f"wv{cb}") for cb in range(CB)]

            # DMA queues (only sync/scalar/gpsimd may start DMAs): wqk8
            # on sync; xT8 chunk-0 + bf16-x chunk-0 + wv on scalar; bq +
            # later xT8 chunks + wo on gpsimd.
            nc.sync.dma_start(WT8h[0][:], wqk8_ap[:, 0:4])
            nc.sync.dma_start(WT8h[1][:], wqk8_ap[:, 4:8])
            nc.scalar.dma_start(XT8[0][:], xT8_ap[0])
            alloc_xtb(0, nc.gpsimd)
            bq_t = [const_pool.tile([128, 1], F32, name=f"bq{n}") for n in range(4)]
            for n in range(4):
                nc.gpsimd.dma_start(bq_t[n][:], bq_ap[n])
            for q in range(1, TQ_CH):
                nc.gpsimd.dma_start(XT8[q][:], xT8_ap[q])
            for cb in range(CB):
                nc.scalar.dma_start(WV[cb][:], wv_ap[cb])
            for g in range(4):
                nc.gpsimd.dma_start(WO[g][:], wo_ap[g])

            # constants: triangular mask (valid = key_i <= query_j); [1,128]
            # head-pair broadcast masks ones_a (cols 0-63) / ones_b (64-127).
            trif = const_pool.tile([128, 128], F32)
            make_upper_triangular(nc, trif, val=1.0, diag=True)
            tri = const_pool.tile([128, 128], BF16)
            nc.gpsimd.tensor_copy(tri[:], trif[:])
            oaf = const_pool.tile([1, 128], F32)
            nc.gpsimd.memset(oaf, 0.0)
            nc.gpsimd.memset(oaf[0:1, 0:64], 1.0)
            obf = const_pool.tile([1, 128], F32)
            nc.gpsimd.memset(obf, 0.0)
            nc.gpsimd.memset(obf[0:1, 64:128], 1.0)
            ones_a = const_pool.tile([1, 128], BF16)
            nc.gpsimd.tensor_copy(ones_a[:], oaf[:])
            ones_b = const_pool.tile([1, 128], BF16)
            nc.gpsimd.tensor_copy(ones_b[:], obf[:])
            onesf = const_pool.tile([128, HL], F32)
            nc.gpsimd.memset(onesf, 1.0)
            for t in range(TK_TILES):
                nc.gpsimd.tensor_copy(V[t][:, :, D], onesf[:])

            # ---------- projection / o-projection filler machinery ----------
            fill_q = []          # ('qkt', n, q1) | ('v', tt), item = 8 matmuls

            def emit_fill_item():
                kind = fill_q[0][0]
                ps = ps_p_pool.tile([128, 512], F32, name="psp")
                if kind == 'qkt':
                    _, n, q1 = fill_q.pop(0)
                    for cbp in range(4):
                        nc.tensor.matmul(
                            ps[:], WT8h[n // 4][:, n % 4, cbp],
                            XT8[q1][:, cbp],
                            start=(cbp == 0), stop=(cbp == 3),
                            perf_mode=mybir.MatmulPerfMode.DoubleRow,
                        )
                    dst = QKT[n][:, q1 * 512:(q1 + 1) * 512]
                    if n < 4:
                        nc.vector.tensor_scalar(
                            dst, ps[:], PROJ_SCALE, bq_t[n][:],
                            mybir.AluOpType.mult, mybir.AluOpType.add,
                        )
                    else:
                        nc.vector.tensor_scalar_mul(dst, ps[:], PROJ_SCALE)
                else:
                    _, tt = fill_q.pop(0)
                    q1, c0 = tt // 4, (tt % 4) * 128
                    for cb in range(CB):
                        nc.tensor.matmul(
                            ps[:], XTB[q1][cb][:, c0:c0 + 128], WV[cb][:],
                            start=(cb == 0), stop=(cb == CB - 1),
                        )
                    nc.vector.tensor_copy(
                        V[tt][:, :, 0:D],
                        ps[:].rearrange("p (h d) -> p h d", h=HL),
                    )

            def chunk_items(c):
                # QK col-blocks in first-use order, then the chunk's V tiles
                return ([('qkt', n, c) for n in (0, 4, 1, 5, 2, 6, 3, 7)]
                        + [('v', tt) for tt in range(4 * c, 4 * c + 4)])

            oproj_items = []     # (tt, half) ready once their chunk normalized

            def emit_oproj_item():
                tt, half = oproj_items.pop(0)
                n0 = half * 512
                psp = ps_p_pool.tile([128, 512], F32, name="psp")
                for g in range(4):
                    nc.tensor.matmul(
                        psp[:],
                        OT[g][:, tt * 128:(tt + 1) * 128],
                        WO[g][:, n0:n0 + 512],
                        start=(g == 0), stop=(g == 3),
                    )
                ob = ostg_pool.tile([128, 512], F16, name="ob")
                nc.vector.tensor_copy(ob[:], psp[:])
                nc.sync.dma_start(
                    out_ap[tt * 128:(tt + 1) * 128, n0:n0 + 512], ob[:]
                )

            def stage1(fin):
                """pass2(fin) just ended: evacuate + prep 1/den; frees pso."""
                q0p, hpp, ta, ca, tb, cb2 = fin
                cols = slice(q0p * 512, q0p * 512 + 512)
                # denominator chain first: the reciprocal is the tail's
                # critical path; the OT rows aren't needed until stage2
                sd = rb_pool.tile([1, 1024], F32, name="sd")
                nc.vector.tensor_copy(sd[0:1, 0:512], ta[64:65, ca:ca + 512])
                nc.vector.tensor_copy(sd[0:1, 512:1024],
                                      tb[64:65, cb2:cb2 + 512])
                rf = rb_pool.tile([1, 1024], F32, name="rf")
                nc.vector.reciprocal_approx_fast(rf[:], sd[:])
                rb8 = rb2_pool.tile([1, 1024], BF16, name="rb8")
                nc.vector.tensor_copy(rb8[:], rf[:])
                nc.vector.tensor_copy(OT[hpp][0:64, cols],
                                      ta[0:64, ca:ca + 512])
                nc.vector.tensor_copy(OT[hpp][64:128, cols],
                                      tb[0:64, cb2:cb2 + 512])
                return (q0p, hpp, rb8[0:1, 0:512], rb8[0:1, 512:1024])

            def stage2(fin2):
                """one group later: broadcast 1/den and normalize OT."""
                q0p, hpp, rba, rbb = fin2
                cols = slice(q0p * 512, q0p * 512 + 512)
                psb = ps_p_pool.tile([128, 512], F32, name="psp")
                nc.tensor.matmul(psb[:], ones_a[:], rba[:],
                                 start=True, stop=False)
                nc.tensor.matmul(psb[:], ones_b[:], rbb[:],
                                 start=False, stop=True)
                nc.vector.tensor_mul(OT[hpp][:, cols], OT[hpp][:, cols],
                                     psb[:])
                if hpp == 3:   # chunk q0p fully normalized
                    oproj_items.extend(
                        (tt, half)
                        for tt in range(4 * q0p, 4 * q0p + 4)
                        for half in range(2)
                    )

            cs_total, cs_idx, fill_emitted = 1, 0, 0
            # ---------------- prefix: chunk-0 projection ----------------
            fill_q.extend(chunk_items(0))
            while fill_q:
                emit_fill_item()

            # ---------------- merged main loop (chunks ascending) ----------
            groups = [(q0, hp) for q0 in range(TQ_CH) for hp in range(4)]
            prev = None   # (q0, hp, Ea, Eb) whose pass2 runs this iteration
            fin2 = None   # stage1 output awaiting stage2

            for cur in groups + [None]:
                ntk_c = 4 * (cur[0] + 1) if cur is not None else 0
                ntk_p = 4 * (prev[0] + 1) if prev is not None else 0
                if cur is not None:
                    q0, hp = cur
                    qt, kt = QKT[hp], QKT[4 + hp]
                    tq0 = q0 * 512
                    Ea, Eb = {}, {}
                    if hp == 0:
                        cs_total = 16 * (q0 + 1)
                        cs_idx = 0
                        fill_emitted = 0
                        if q0 + 1 < TQ_CH:
                            alloc_xtb(q0 + 1, nc.gpsimd)
                            fill_q.extend(chunk_items(q0 + 1))
                pso = None
                for j in range(max(ntk_c, ntk_p)):
                    if cur is not None and j < ntk_c:
                        r = j - q0 * 4
                        j0 = r * 128 if r >= 0 else 0
                        pss = ps_s_pool.tile([128, 1024], F32, name="pss")
                        nc.tensor.matmul(
                            pss[:, j0:512],
                            kt[0:64, j * 128:(j + 1) * 128],
                            qt[0:64, tq0 + j0:tq0 + 512],
                            start=True, stop=True,
                        )
                        nc.tensor.matmul(
                            pss[:, 512 + j0:1024],
                            kt[64:128, j * 128:(j + 1) * 128],
                            qt[64:128, tq0 + j0:tq0 + 512],
                            start=True, stop=True,
                        )
                        epool = e_pool if j < 12 else e2_pool
                        e_ab = epool.tile([128, 1024], BF16, name=f"eab{j}")
                        Ea[j] = Eb[j] = e_ab
                        # one wide activation covering both heads' S
                        # (spans both PSUM banks of the pss tile); on
                        # diagonal steps the [512:512+j0] middle is stale
                        # junk that is exp'd but never read
                        nc.scalar.activation(
                            e_ab[:, j0:1024], pss[:, j0:1024],
                            mybir.ActivationFunctionType.Exp,
                            scale=float(D) ** -0.5,
                        )
                        if r >= 0:
                            nc.gpsimd.tensor_mul(
                                e_ab[:, j0:j0 + 128], e_ab[:, j0:j0 + 128],
                                tri[:]
                            )
                            nc.gpsimd.tensor_mul(
                                e_ab[:, 512 + j0:512 + j0 + 128],
                                e_ab[:, 512 + j0:512 + j0 + 128], tri[:]
                            )
                    if cur is not None:
                        cs_idx += 1
                    if fill_q and (cur is None or
                                   fill_emitted * cs_total < cs_idx * 12):
                        emit_fill_item()
                        fill_emitted += 1
                    elif oproj_items:
                        emit_oproj_item()
                    if prev is not None and j < ntk_p:
                        q0p, hpp, pEa, pEb = prev
                        if j == 0:
                            if (q0p, hpp) == groups[-1]:
                                # ps_p banks have been idle since the last
                                # broadcast -> no wait, unlike ps_o/ps_s
                                pso = (
                                    ps_p_pool.tile([128, 512], F32,
                                                   name="psp"), 0,
                                    ps_p_pool.tile([128, 512], F32,
                                                   name="psp"), 0,
                                )
                            else:
                                pso = (
                                    ps_o_pool.tile([65, 512], F32,
                                                   name="pso_a"), 0,
                                    ps_o_pool.tile([65, 512], F32,
                                                   name="pso_b"), 0,
                                )
                        ta, ca, tb, cb2 = pso
                        rp = j - q0p * 4
                        j0p = rp * 128 if rp >= 0 else 0
                        nc.tensor.matmul(
                            ta[0:65, ca + j0p:ca + 512],
                            V[j][:, 2 * hpp, :],
                            pEa[j][:, j0p:512],
                            start=(j == 0), stop=(j == ntk_p - 1),
                        )
                        nc.tensor.matmul(
                            tb[0:65, cb2 + j0p:cb2 + 512],
                            V[j][:, 2 * hpp + 1, :],
                            pEb[j][:, 512 + j0p:1024],
                            start=(j == 0), stop=(j == ntk_p - 1),
                        )
                new_fin2 = None
                if prev is not None:
                    q0p, hpp, _, _ = prev
                    new_fin2 = stage1((q0p, hpp) + pso)
                if fin2 is not None:
                    stage2(fin2)
                fin2 = new_fin2
                prev = (q0, hp, Ea, Eb) if cur is not None else None
            stage2(fin2)
            while oproj_items:
                emit_oproj_item()

    nc.compile()
    return nc


def _prep_core_inputs(hidden_state, qkv_w, qkv_b, o_w, b, hg):
    """Build the per-core input map for batch b, head group hg."""
    import ml_dtypes
    bf16 = ml_dtypes.bfloat16
    s = slice(hg * HD, (hg + 1) * HD)
    wq = qkv_w[:, 0 * C:1 * C][:, s]          # [C, 512]
    wk = qkv_w[:, 1 * C:2 * C][:, s]          # [C, 512]
    wv = qkv_w[:, 2 * C:3 * C][:, s]          # [C, 512]
    bq = qkv_b[0 * C:1 * C][s]                # [512]

    wqk = np.concatenate([wq, wk], axis=1)    # [C, 1024]
    f8 = mybir.dt.np(mybir.dt.float8e4)
    # paired-contraction fp8 layouts: c = cbp*256 + j*128 + p
    wqk8 = np.ascontiguousarray(
        (wqk * 2048.0).reshape(4, 2, 128, 8, 128).transpose(2, 3, 0, 1, 4)
    ).astype(f8)
    bq_r = np.ascontiguousarray(bq.reshape(4, 128, 1))
    wv_r = np.ascontiguousarray(wv.reshape(CB, 128, HD))
    # o_w rows for this head group, regrouped [g, 128, C] in head-pair order
    wo = o_w[hg * HD:(hg + 1) * HD, :]        # [512, C]
    wo_r = np.ascontiguousarray(wo.reshape(4, 128, C))

    xT = np.ascontiguousarray(hidden_state[b].T)  # [C, T]
    xT8 = np.ascontiguousarray(
        (xT * 16.0).reshape(4, 2, 128, TQ_CH, 512).transpose(3, 2, 0, 1, 4)
    ).astype(f8)
    return {
        "xT": xT.astype(bf16),
        "xT8": xT8,
        "wqk8": wqk8,
        "bq": bq_r.astype(np.float32),
        "wv": wv_r.astype(bf16),
        "wo": wo_r.astype(bf16),
    }


def _ensure_profile_hook():
    """Register the NTFF profiling hook that this container's antenv lacks."""
    import types
    try:
        from antenv.axon_hooks import get_axon_ntff_profile_hook  # noqa: F401
        return
    except ImportError:
        pass
    try:
        import antenv
        from trn_agent_boot.trn_boot import _ntff_profile_via_ctypes
        hook = {"h": _ntff_profile_via_ctypes("/opt/axon/libaxon_pjrt.so")}
        mod = types.ModuleType("antenv.axon_hooks")
        mod.set_axon_ntff_profile_hook = lambda h: hook.__setitem__("h", h)
        mod.get_axon_ntff_profile_hook = lambda: hook["h"]
        sys.modules["antenv.axon_hooks"] = mod
        antenv.axon_hooks = mod
    except Exception as e:  # profiling is best-effort
        print(f"profile hook setup failed: {e}", flush=True)


def kernel(hidden_state, qkv_w, qkv_b, o_w, o_b):
    global _compiled
    hidden_state = np.asarray(hidden_state, dtype=np.float32)
    qkv_w = np.asarray(qkv_w, dtype=np.float32)
    qkv_b = np.asarray(qkv_b, dtype=np.float32)
    o_w = np.asarray(o_w, dtype=np.float32)
    o_b = np.asarray(o_b, dtype=np.float32)

    if _compiled is None:
        _compiled = _build()
    nc = _compiled

    in_maps = []
    for core in range(N_CORES):
        b, hg = core // 2, core % 2
        in_maps.append(_prep_core_inputs(hidden_state, qkv_w, qkv_b, o_w, b, hg))

    global LAST_EXEC_NS, LAST_TRACE
    kw = {}
    if TRACE:
        import tempfile
        _ensure_profile_hook()
        kw = dict(trace=True, tmpdir=tempfile.mkdtemp(prefix="bass_attn_trace_"))
    res = run_bass_kernel_spmd(nc, in_maps, core_ids=list(range(N_CORES)), **kw)
    LAST_EXEC_NS = res.exec_time_ns
    LAST_TRACE = res.instructions_and_trace

    # host-side gather: sum the two head-group partials per batch and add the
    # affine correction (v-bias pushed through Wo, plus o-bias).
    bv = qkv_b[2 * C:3 * C]                   # [C]
    corr = (bv @ o_w + o_b).astype(np.float32)
    out = np.empty((B, T, C), dtype=np.float32)
    for b in range(B):
        p0 = res.results[2 * b]["out_p"].astype(np.float32)
        p1 = res.results[2 * b + 1]["out_p"].astype(np.float32)
        out[b] = p0 + p1 + corr
    return out
